# revision 4
# baseline (speedup 1.0000x reference)
"""CustomSAGEConv on 8 Trainium2 NeuronCores.

out = x @ W_self.T + b_self + segment_mean(msgs[row], col)
where msgs = x @ W_msg.T + b_msg.

Since the message projection is linear, it commutes with the mean:
  agg = (segment_sum(x[row], col) / max(deg,1)) @ W_msg.T + (deg>0)*b_msg
so the device only needs the raw-feature segment sum, then two small
[64,64] projections per 128-node block.

Sharding: destination nodes are padded to 50176 = 8 cores x 49 groups x 128
nodes. Edges are bucketed by destination group on the host (this is the edge
partitioning step), so each core exclusively owns its 49 output groups and no
cross-core reduction is needed. Per group, the segment sum is computed as a
one-hot matmul accumulated in PSUM:
  psum[64f, 128d] += xg[128e, 64f].T @ onehot(dest)[128e, 128d]
with source rows brought in by dma_gather. dma_gather indices are int16, so
x is split at row 32768 into lo/hi tables and each group's edges are
partitioned into a lo list and a hi list (two gathers per group).
"""

import sys

for _p in ("/opt/trn_rl_repo", "/root/.axon_site/_ro/trn_rl_repo"):
    if _p not in sys.path:
        sys.path.insert(0, _p)

import numpy as np

P = 128
D = 64
NC = 8
N_NODES = 50000
SPLIT = 32768

_CACHE = {}


def _ceil_div(a, b):
    return (a + b - 1) // b


def _build_bass(T_A, T_B, GPC, n_lo, n_hi, with_bias, repeat=1):
    import concourse.mybir as mybir
    import concourse.tile as tile
    from concourse import bacc

    T = T_A + T_B
    NPC = GPC * P  # nodes per core

    nc = bacc.Bacc(num_swdge_queues=4)
    f32 = mybir.dt.float32
    x_lo = nc.declare_dram_parameter("x_lo", [n_lo, D], f32, isOutput=False)
    x_hi = nc.declare_dram_parameter("x_hi", [n_hi, D], f32, isOutput=False)
    xT = nc.declare_dram_parameter("xT", [D, NPC], f32, isOutput=False)
    idxA = nc.declare_dram_parameter("idxA", [P, GPC * T_A * 8], mybir.dt.int16, isOutput=False)
    idxB = nc.declare_dram_parameter("idxB", [P, GPC * T_B * 8], mybir.dt.int16, isOutput=False)
    dest = nc.declare_dram_parameter("dest", [P, GPC * T], f32, isOutput=False)
    rv = nc.declare_dram_parameter("rv", [P, GPC], f32, isOutput=False)
    Wm = nc.declare_dram_parameter("Wm", [D, D], f32, isOutput=False)
    Ws = nc.declare_dram_parameter("Ws", [D, D], f32, isOutput=False)
    if with_bias:
        bias = nc.declare_dram_parameter("bias", [NPC, D], f32, isOutput=False)
    out = nc.declare_dram_parameter("out", [NPC, D], f32, isOutput=True)

    with tile.TileContext(nc) as tc:
        with (
            tc.tile_pool(name="const", bufs=1) as cpool,
            tc.tile_pool(name="gather", bufs=3) as gpool,
            tc.tile_pool(name="oh", bufs=8) as ohpool,
            tc.tile_pool(name="small", bufs=3) as spool,
            tc.tile_pool(name="psum1", bufs=2, space="PSUM") as p1pool,
            tc.tile_pool(name="psum2", bufs=2, space="PSUM") as p2pool,
        ):
            iota_i = cpool.tile([P, P], mybir.dt.int32)
            nc.gpsimd.iota(iota_i[:], pattern=[[1, P]], base=0, channel_multiplier=0)
            iota_f = cpool.tile([P, P], f32)
            nc.vector.tensor_copy(iota_f[:], iota_i[:])

            idxA_sb = cpool.tile([P, GPC * T_A * 8], mybir.dt.int16)
            nc.sync.dma_start(out=idxA_sb[:], in_=idxA[:])
            idxB_sb = cpool.tile([P, GPC * T_B * 8], mybir.dt.int16)
            nc.sync.dma_start(out=idxB_sb[:], in_=idxB[:])
            dest_sb = cpool.tile([P, GPC * T], f32)
            nc.sync.dma_start(out=dest_sb[:], in_=dest[:])
            rv_sb = cpool.tile([P, GPC], f32)
            nc.sync.dma_start(out=rv_sb[:], in_=rv[:])
            Wm_sb = cpool.tile([D, D], f32)
            nc.sync.dma_start(out=Wm_sb[:], in_=Wm[:])
            Ws_sb = cpool.tile([D, D], f32)
            nc.sync.dma_start(out=Ws_sb[:], in_=Ws[:])

            for g in range(GPC * repeat):
                g = g % GPC
                xga = gpool.tile([P, T_A, D], f32, tag="xga")
                nc.gpsimd.dma_gather(
                    out_ap=xga[:],
                    in_ap=x_lo[:],
                    idxs_ap=idxA_sb[:, g * T_A * 8:(g + 1) * T_A * 8],
                    num_idxs=T_A * P,
                    num_idxs_reg=T_A * P,
                    elem_size=D,
                    single_packet=False,
                    queue_num=g % 4,
                )
                xgb = gpool.tile([P, T_B, D], f32, tag="xgb")
                nc.gpsimd.dma_gather(
                    out_ap=xgb[:],
                    in_ap=x_hi[:],
                    idxs_ap=idxB_sb[:, g * T_B * 8:(g + 1) * T_B * 8],
                    num_idxs=T_B * P,
                    num_idxs_reg=T_B * P,
                    elem_size=D,
                    single_packet=False,
                    queue_num=(g + 2) % 4,
                )
                xtb = spool.tile([D, P], f32, tag="xtb")
                nc.sync.dma_start(out=xtb[:], in_=xT[:, g * P:(g + 1) * P])

                psum1 = p1pool.tile([D, P], f32)
                for t in range(T):
                    oh = ohpool.tile([P, P], f32)
                    nc.vector.tensor_tensor(
                        out=oh[:],
                        in0=iota_f[:],
                        in1=dest_sb[:, g * T + t:g * T + t + 1].to_broadcast([P, P]),
                        op=mybir.AluOpType.is_equal,
                    )
                    src = xga[:, t, :] if t < T_A else xgb[:, t - T_A, :]
                    nc.tensor.matmul(
                        psum1[:],
                        lhsT=src,
                        rhs=oh[:],
                        start=(t == 0),
                        stop=(t == T - 1),
                    )

                aggT = spool.tile([D, P], f32, tag="aggT")
                nc.scalar.copy(out=aggT[:], in_=psum1[:])

                pmsg = p2pool.tile([P, D], f32, tag="pmsg")
                nc.tensor.matmul(pmsg[:], lhsT=aggT[:], rhs=Wm_sb[:], start=True, stop=True)
                pself = p2pool.tile([P, D], f32, tag="pself")
                nc.tensor.matmul(pself[:], lhsT=xtb[:], rhs=Ws_sb[:], start=True, stop=True)

                out_sb = spool.tile([P, D], f32, tag="out_sb")
                nc.vector.tensor_tensor(
                    out=out_sb[:],
                    in0=pmsg[:],
                    in1=rv_sb[:, g:g + 1].to_broadcast([P, D]),
                    op=mybir.AluOpType.mult,
                )
                nc.vector.tensor_tensor(
                    out=out_sb[:], in0=out_sb[:], in1=pself[:], op=mybir.AluOpType.add
                )
                if with_bias:
                    bias_sb = spool.tile([P, D], f32, tag="bias_sb")
                    nc.sync.dma_start(out=bias_sb[:], in_=bias[g * P:(g + 1) * P, :])
                    nc.vector.tensor_tensor(
                        out=out_sb[:], in0=out_sb[:], in1=bias_sb[:],
                        op=mybir.AluOpType.add,
                    )
                nc.sync.dma_start(out=out[g * P:(g + 1) * P, :], in_=out_sb[:])
    nc.compile()
    return nc


def _wrap_idx(slots):
    """[G, S] per-slot gather indices -> [G, 128, S//16*... ] int16 wrapped.

    dma_gather reads index i from partition i%16, column i//16 (replicated
    across the 8 Q7 cores' 16-partition slices).
    """
    G, S = slots.shape
    w = slots.reshape(G, S // 16, 16).transpose(0, 2, 1)  # [G, 16, S//16]
    return np.tile(w, (1, 8, 1)).astype(np.int16)  # [G, 128, S//16]


def prepare(x, edge_index, W_msg, b_msg, W_self, b_self):
    x = np.asarray(x, dtype=np.float32)
    edge_index = np.asarray(edge_index)
    W_msg = np.asarray(W_msg, dtype=np.float32)
    W_self = np.asarray(W_self, dtype=np.float32)
    b_msg = np.asarray(b_msg, dtype=np.float32)
    b_self = np.asarray(b_self, dtype=np.float32)

    n = x.shape[0]
    GPC = _ceil_div(n, P * NC)
    G = NC * GPC
    NPAD = G * P
    NPC = GPC * P

    row = edge_index[0].astype(np.int64)
    col = edge_index[1].astype(np.int64)
    grp = (col // P).astype(np.int64)
    isB = row >= SPLIT

    # per-(group, half) counts and slot positions
    cntA = np.bincount(grp[~isB], minlength=G)
    cntB = np.bincount(grp[isB], minlength=G)
    T_A = max(1, _ceil_div(int(cntA.max()), P))
    T_B = max(1, _ceil_div(int(cntB.max()), P))
    T = T_A + T_B

    deg = np.bincount(col, minlength=NPAD).astype(np.int64)
    rv_full = (1.0 / np.maximum(deg, 1)).astype(np.float32)

    # slot tables: [G, T_*128] gather index (0-padded) + dest id (300-padded)
    slotsA = np.zeros((G, T_A * P), dtype=np.int64)
    destA = np.full((G, T_A * P), 300.0, dtype=np.float32)
    slotsB = np.zeros((G, T_B * P), dtype=np.int64)
    destB = np.full((G, T_B * P), 300.0, dtype=np.float32)

    for slots, destv, mask, base in (
        (slotsA, destA, ~isB, 0),
        (slotsB, destB, isB, SPLIT),
    ):
        r = row[mask]
        c = col[mask]
        g_of = grp[mask]
        o = np.argsort(g_of, kind="stable")
        r, c, g_of = r[o], c[o], g_of[o]
        cnt = np.bincount(g_of, minlength=G)
        starts = np.zeros(G + 1, dtype=np.int64)
        np.cumsum(cnt, out=starts[1:])
        pos = np.arange(len(r)) - starts[g_of]
        slots[g_of, pos] = r - base
        destv[g_of, pos] = (c - g_of * P).astype(np.float32)

    idxA_w = _wrap_idx(slotsA)  # [G, 128, T_A*8]
    idxB_w = _wrap_idx(slotsB)

    # dest layout: slot i of group -> partition i%128, tile i//128
    dest_all = np.concatenate([destA, destB], axis=1)  # [G, T*128]
    dest_pt = dest_all.reshape(G, T, P).transpose(0, 2, 1)  # [G, 128, T]

    x_pad = np.zeros((NPAD, D), dtype=np.float32)
    x_pad[:n] = x
    x_lo = np.ascontiguousarray(x_pad[:SPLIT])
    x_hi = np.ascontiguousarray(x_pad[SPLIT:])
    n_lo, n_hi = x_lo.shape[0], x_hi.shape[0]

    WmT = np.ascontiguousarray(W_msg.T)
    WsT = np.ascontiguousarray(W_self.T)

    with_bias = bool(b_msg.any() or b_self.any())
    if with_bias:
        ind = (deg > 0).astype(np.float32)
        bias_full = b_self[None, :] + ind[:, None] * b_msg[None, :]

    in_maps = []
    for c in range(NC):
        gs = slice(c * GPC, (c + 1) * GPC)
        m = {
            "x_lo": x_lo,
            "x_hi": x_hi,
            "xT": np.ascontiguousarray(x_pad[c * NPC:(c + 1) * NPC].T),
            "idxA": np.ascontiguousarray(
                idxA_w[gs].transpose(1, 0, 2).reshape(P, GPC * T_A * 8)
            ),
            "idxB": np.ascontiguousarray(
                idxB_w[gs].transpose(1, 0, 2).reshape(P, GPC * T_B * 8)
            ),
            "dest": np.ascontiguousarray(
                dest_pt[gs].transpose(1, 0, 2).reshape(P, GPC * T)
            ),
            "rv": np.ascontiguousarray(
                rv_full[c * NPC:(c + 1) * NPC].reshape(GPC, P).T
            ),
            "Wm": WmT,
            "Ws": WsT,
        }
        if with_bias:
            m["bias"] = np.ascontiguousarray(bias_full[c * NPC:(c + 1) * NPC])
        in_maps.append(m)

    meta = (T_A, T_B, GPC, n_lo, n_hi, with_bias, n)
    return meta, in_maps


def kernel(x, edge_index, W_msg, b_msg, W_self, b_self, _trace=False):
    from concourse.bass_utils import run_bass_kernel_spmd

    meta, in_maps = prepare(x, edge_index, W_msg, b_msg, W_self, b_self)
    T_A, T_B, GPC, n_lo, n_hi, with_bias, n = meta

    key = meta[:-1]
    if key not in _CACHE:
        _CACHE[key] = _build_bass(T_A, T_B, GPC, n_lo, n_hi, with_bias)
    nc = _CACHE[key]

    res = run_bass_kernel_spmd(nc, in_maps, list(range(NC)), trace=_trace)
    full = np.concatenate([res.results[c]["out"] for c in range(NC)], axis=0)
    out = np.ascontiguousarray(full[:n]).astype(np.float32, copy=False)
    if _trace:
        return out, res
    return out
